# revision 4
# baseline (speedup 1.0000x reference)
"""DifferentialAttention Trainium2 Bass kernel.

Sharding: 8 cores = 2 batches x 4 head-pairs (2 heads each).
Per core (SPMD, same program, different data):
  phase A: q/k/v projections as fp8e4 DoubleRow matmuls with hi+lo error
           compensation (x and w host-split into e4m3 hi + e4m3 residual;
           3 of the 4 cross terms computed -> ~bf16 accuracy at 2x PE rate),
           RMS-norm, PE transpose into [d, t] layout. The reference's
           "rotary" uses the head index as its position, giving a constant
           orthogonal rotation per head identical for q and k -- it cancels
           in q.k^T and is skipped entirely.
  phase B: differential causal attention per head in bf16, exp without
           max-subtract (|S*scale| <= sqrt(128), safe in f32), softmax
           denominator via ones-matmuls over the exp'd score tiles,
           normalization applied to O^T after a gpsimd partition broadcast.
  phase C: output projection, also hi/lo fp8 DoubleRow (oT quantized
           on-chip on DVE, wo host-split); all scale factors are powers of
           two folded into one final PSUM->SBUF copy scale. y emitted bf16;
           per-core partial y summed over head-pair cores on host.
"""

import json
import os
import sys
import tempfile
from contextlib import ExitStack

import numpy as np

sys.path.insert(0, "/opt/trn_rl_repo")

import ml_dtypes  # noqa: E402

import concourse.bass as bass  # noqa: E402
import concourse.mybir as mybir  # noqa: E402
import concourse.tile as tile  # noqa: E402
from concourse import bacc, bass_utils  # noqa: E402
from concourse.masks import make_identity  # noqa: E402

B, T, C = 2, 2048, 2048
NH, HD, HH = 8, 256, 128
LAMBDA_INIT = 0.2
RMS_EPS = 1.1920929e-07
SCALE = float(1.0 / np.sqrt(np.float32(HH)))

F32 = mybir.dt.float32
BF16 = mybir.dt.bfloat16
FP8 = mybir.dt.float8e4
NPBF16 = ml_dtypes.bfloat16
NPFP8 = ml_dtypes.float8_e4m3
DR = mybir.MatmulPerfMode.DoubleRow

NM = T // 128          # 16 m-tiles (t blocks)
NK = C // 128          # 16 k-tiles (c blocks)
NKP = NK // 2          # 8 DoubleRow k-pairs
NCH = T // 512         # 4 tq chunks

# power-of-two fp8 scale factors (exact in fp32)
SX = 16.0              # x pre-scale
SW = 512.0             # wq/wk/wv pre-scale
SWO = 512.0            # wo pre-scale
SO = 1.0 / 256.0       # on-chip oT quantize scale
# y_psum = (SX*SW * o_true)*SO @ (SWO * 0.8*wo) => unscale:
YSCALE = float(1.0 / (SX * SW * SO * SWO))

_ACT_TABLES_DONE = False


def _setup_act_tables():
    """Reorder act_info so `natural_log_exp_and_others` is the first table:
    it covers every ACT func we use (square, ln, exp, copy), so the greedy
    table selector stays on one table instead of thrashing exp<->ln loads."""
    global _ACT_TABLES_DONE
    if _ACT_TABLES_DONE:
        return
    from neuronxcc.driver.Job import Job  # noqa: PLC0415
    from neuronxcc.driver.jobs.support.FindActInfo import (  # noqa: PLC0415
        findActInfoFile,
    )

    src = findActInfoFile(Job.getPackageDir(), "gen3")
    srcdir = os.path.dirname(src)
    with open(src) as f:
        info = json.load(f)
    info["act_func_sets"].sort(
        key=lambda s: s["name"] != "natural_log_exp_and_others")
    dstdir = os.path.join(tempfile.gettempdir(), "act_info_nlexp_first")
    os.makedirs(dstdir, exist_ok=True)
    for name in os.listdir(srcdir):
        dst = os.path.join(dstdir, name)
        if not os.path.exists(dst):
            try:
                os.symlink(os.path.join(srcdir, name), dst)
            except OSError:
                pass
    act_path = os.path.join(dstdir, "act_info.json")
    with open(act_path, "w") as f:
        json.dump(info, f)
    os.environ["BASS_ACT_ROOT_JSON_PATH"] = act_path

    import concourse.hw_specs as hw_specs  # noqa: PLC0415

    def patched(module_arch):
        return {
            e["name"]: {
                mybir.ActivationFunctionType.from_pwp(v) for v in e["act"]
            }
            for e in info["act_func_sets"]
        }

    hw_specs.get_activation_tables = patched
    bacc.get_activation_tables = patched
    _ACT_TABLES_DONE = True


def _bcast_cols(ap2d, col0, nblk, inner):
    """[128, nblk, inner] view of columns col0..col0+nblk of a [128, n] tile,
    each column replicated `inner` times along a 0-stride inner dim."""
    return bass.AP(
        tensor=ap2d.tensor,
        offset=ap2d.offset + col0,
        ap=[ap2d.ap[0], [1, nblk], [0, inner]],
    )


def _body(tc, aps):
    nc = tc.nc
    (xTh, xTl, wqTh, wqTl, wkTh, wkTl, wvTh, wvTl,
     woTh, woTl, tri, ones, neglam, y) = aps

    xTh_r = xTh.rearrange("(k p) t -> p k t", p=128)     # [128, 16, 2048]
    xTl_r = xTl.rearrange("(k p) t -> p k t", p=128)
    wqTh_r = wqTh.rearrange("(k p) n -> p k n", p=128)   # [128, 16, 512]
    wqTl_r = wqTl.rearrange("(k p) n -> p k n", p=128)
    wkTh_r = wkTh.rearrange("(k p) n -> p k n", p=128)
    wkTl_r = wkTl.rearrange("(k p) n -> p k n", p=128)
    wvTh_r = wvTh.rearrange("(k p) n -> p k n", p=128)
    wvTl_r = wvTl.rearrange("(k p) n -> p k n", p=128)
    woTh_r = woTh.rearrange("(k p) n -> p k n", p=128)   # [128, 4, 2048]
    woTl_r = woTl.rearrange("(k p) n -> p k n", p=128)

    with ExitStack() as octx:
        # ---- persistent tiles (live across phases) ----
        persist = octx.enter_context(tc.tile_pool(name="persist", bufs=1))
        qkT_all = persist.tile([128, 8, T], BF16)    # seg h*4 + [q1,q2,k1,k2]
        v_all = persist.tile([128, NM, 512], BF16)   # [t(P) per m, e: h0|h1]
        tri_sb = persist.tile([128, 128], BF16)
        ones_sb = persist.tile([128, 1], BF16)
        neglam_sb = persist.tile([128, 1], F32)
        ident_sb = persist.tile([128, 128], BF16)

        nc.sync.dma_start(out=tri_sb, in_=tri)
        nc.sync.dma_start(out=ones_sb, in_=ones)
        nc.sync.dma_start(out=neglam_sb, in_=neglam)
        make_identity(nc, ident_sb)

        # segment mapping: qkT_all viewed [128, h, 4, T]; q -> [:, :, 0:2],
        # k -> [:, :, 2:4]; block order within a group is (h0b1,h0b2,h1b1,h1b2)
        qkT_v = qkT_all.rearrange("p (h f) t -> p h f t", h=2)

        # ================= Phase A: projections =================
        with ExitStack() as actx:
            wpool = actx.enter_context(tc.tile_pool(name="wpool", bufs=1))
            xpool = actx.enter_context(tc.tile_pool(name="xpool", bufs=1))
            psA = actx.enter_context(
                tc.tile_pool(name="psA", bufs=6, space="PSUM"))
            psT = actx.enter_context(
                tc.tile_pool(name="psT", bufs=2, space="PSUM"))
            sbA = actx.enter_context(tc.tile_pool(name="sbA", bufs=2))

            wq_sb = [wpool.tile([128, NK, 512], FP8, name=f"wq_sb{i}")
                     for i in range(2)]
            wk_sb = [wpool.tile([128, NK, 512], FP8, name=f"wk_sb{i}")
                     for i in range(2)]
            wv_sb = [wpool.tile([128, NK, 512], FP8, name=f"wv_sb{i}")
                     for i in range(2)]
            x_sb = [xpool.tile([128, NK, T], FP8, name=f"x_sb{i}")
                    for i in range(2)]
            # hi bytes first: hi-hi matmuls of early m-tiles can start after
            # ~2.6MB instead of the full 16.8MB input set.
            for kk in range(NK):
                nc.sync.dma_start(out=wq_sb[0][:, kk, :], in_=wqTh_r[:, kk, :])
                nc.sync.dma_start(out=wk_sb[0][:, kk, :], in_=wkTh_r[:, kk, :])
                nc.sync.dma_start(out=wv_sb[0][:, kk, :], in_=wvTh_r[:, kk, :])
                nc.sync.dma_start(out=x_sb[0][:, kk, 0:512],
                                  in_=xTh_r[:, kk, 0:512])
            for kk in range(NK):
                nc.sync.dma_start(out=wq_sb[1][:, kk, :], in_=wqTl_r[:, kk, :])
                nc.sync.dma_start(out=wk_sb[1][:, kk, :], in_=wkTl_r[:, kk, :])
                nc.sync.dma_start(out=wv_sb[1][:, kk, :], in_=wvTl_r[:, kk, :])
                nc.sync.dma_start(out=x_sb[1][:, kk, 0:512],
                                  in_=xTl_r[:, kk, 0:512])
            for kk in range(NK):
                nc.sync.dma_start(out=x_sb[0][:, kk, 512:T],
                                  in_=xTh_r[:, kk, 512:T])
                nc.sync.dma_start(out=x_sb[1][:, kk, 512:T],
                                  in_=xTl_r[:, kk, 512:T])

            # HAM warmup: identity matmuls fill the DMA-wait head so the PE
            # reaches K=8/8 before the first projection matmul. No DMA deps
            # (identity is gpsimd-generated); the psum scratch slot releases
            # before m=2 needs it.
            wu_ps = psA.tile([128, 128], F32, tag="acc")
            for _ in range(56):
                nc.tensor.matmul(wu_ps, ident_sb, ident_sb,
                                 start=True, stop=True)

            # (x term, w term) for the 3 hi/lo cross terms; lo*lo dropped
            TERMS = ((0, 0), (1, 0), (0, 1))
            for m in range(NM):
                mlo, mhi = m * 128, (m + 1) * 128
                q_ps = psA.tile([128, 512], F32, tag="acc")
                k_ps = psA.tile([128, 512], F32, tag="acc")
                v_ps = psA.tile([128, 512], F32, tag="acc")
                for ti, (xi, wi) in enumerate(TERMS):
                    for kp in range(NKP):
                        lhs = x_sb[xi][:, 2 * kp:2 * kp + 2, mlo:mhi]
                        st = ti == 0 and kp == 0
                        sp = ti == 2 and kp == NKP - 1
                        nc.tensor.matmul(q_ps, lhs,
                                         wq_sb[wi][:, 2 * kp:2 * kp + 2, :],
                                         start=st, stop=sp, perf_mode=DR)
                        nc.tensor.matmul(k_ps, lhs,
                                         wk_sb[wi][:, 2 * kp:2 * kp + 2, :],
                                         start=st, stop=sp, perf_mode=DR)
                        nc.tensor.matmul(v_ps, lhs,
                                         wv_sb[wi][:, 2 * kp:2 * kp + 2, :],
                                         start=st, stop=sp, perf_mode=DR)

                # vacate PSUM quickly: stage q/k to SBUF, v straight out
                qs = sbA.tile([128, 512], F32, tag="qs")
                nc.scalar.copy(qs, q_ps)
                ks = sbA.tile([128, 512], F32, tag="ks")
                nc.vector.tensor_copy(ks, k_ps)
                nc.scalar.copy(v_all[:, m, :], v_ps)

                # sumsq of the 8 blocks: stats cols 0-3 = q, 4-7 = k
                stats = sbA.tile([128, 8], F32, tag="stats")
                sq_scr = sbA.tile([128, 128], F32, tag="sqscr")
                for j in range(8):
                    src = qs if j < 4 else ks
                    off = (j % 4) * 128
                    nc.scalar.activation(
                        sq_scr, src[:, off:off + 128],
                        mybir.ActivationFunctionType.Square,
                        accum_out=stats[:, j:j + 1])
                # rstd = exp(-0.5*ln(mean+eps)) -- ln/exp/square share a table
                veps = sbA.tile([128, 8], F32, tag="veps")
                nc.vector.tensor_scalar(
                    veps, stats, 1.0 / HH, RMS_EPS,
                    mybir.AluOpType.mult, mybir.AluOpType.add)
                lnv = sbA.tile([128, 8], F32, tag="lnv")
                nc.scalar.activation(
                    lnv, veps, mybir.ActivationFunctionType.Ln)
                rstd8 = sbA.tile([128, 8], F32, tag="rstd8")
                nc.scalar.activation(
                    rstd8, lnv, mybir.ActivationFunctionType.Exp, scale=-0.5)

                # normalize (no rotary: the reference's head-axis rotary is a
                # per-head constant orthogonal rotation shared by q and k, so
                # it cancels in q.k^T), batched over the 4 blocks of q then k
                for g, (src, c0) in enumerate(((qs, 0), (ks, 4))):
                    qnr = sbA.tile([128, 512], BF16, tag="qnr")
                    nc.vector.tensor_tensor(
                        qnr.rearrange("p (b i) -> p b i", b=4),
                        src.rearrange("p (b i) -> p b i", b=4),
                        _bcast_cols(rstd8, c0, 4, 128), mybir.AluOpType.mult)

                    tp = psT.tile([128, 512], BF16, tag="tp")
                    for bi in range(4):
                        nc.tensor.transpose(
                            tp[:, bi * 128:(bi + 1) * 128],
                            qnr[:, bi * 128:(bi + 1) * 128], ident_sb)
                    # dst: [128, h(2), br(2), 128] at segment group g
                    nc.vector.tensor_copy(
                        qkT_v[:, :, 2 * g:2 * g + 2, mlo:mhi],
                        tp.rearrange("p (h b i) -> p h b i", h=2, b=2))

        # ================= Phase B: attention =================
        bcpool = octx.enter_context(tc.tile_pool(name="bcpool", bufs=1))
        oT_hi = bcpool.tile([128, 4, T], FP8)        # seg h*2 + etile
        oT_lo = bcpool.tile([128, 4, T], FP8)
        wo_sb = [bcpool.tile([128, 4, T], FP8, name=f"wo_sb{i}")
                 for i in range(2)]
        for kk in range(4):
            nc.sync.dma_start(out=wo_sb[0][:, kk, :], in_=woTh_r[:, kk, :])
        for kk in range(4):
            nc.sync.dma_start(out=wo_sb[1][:, kk, :], in_=woTl_r[:, kk, :])

        with ExitStack() as bctx:
            psS = bctx.enter_context(
                tc.tile_pool(name="psS", bufs=4, space="PSUM"))
            psO = bctx.enter_context(
                tc.tile_pool(name="psO", bufs=4, space="PSUM"))
            sbPT = bctx.enter_context(tc.tile_pool(name="sbPT", bufs=34))
            sbB = bctx.enter_context(tc.tile_pool(name="sbB", bufs=4))

            for h in range(2):
                qT = [qkT_v[:, h, 0, :], qkT_v[:, h, 1, :]]
                kT = [qkT_v[:, h, 2, :], qkT_v[:, h, 3, :]]
                for ch in range(NCH):
                    c0, c1 = ch * 512, ch * 512 + 512
                    ntk = 4 * ch + 4
                    # --- S + exp, both branches ---
                    br_pts = []
                    for br in range(2):
                        pts = []
                        for tkb in range(ntk):
                            n0 = max(c0, tkb * 128)
                            nN = c1 - n0
                            col0 = n0 - c0
                            s_ps = psS.tile([128, 512], F32, tag="s")
                            nc.tensor.matmul(
                                s_ps[:, :nN],
                                kT[br][:, tkb * 128:(tkb + 1) * 128],
                                qT[br][:, n0:c1], start=True, stop=True)
                            pt = sbPT.tile([128, 512], BF16, tag="pt")
                            nc.scalar.activation(
                                pt[:, :nN], s_ps[:, :nN],
                                mybir.ActivationFunctionType.Exp, scale=SCALE)
                            if tkb * 128 >= c0:   # diagonal block
                                nc.vector.tensor_mul(
                                    pt[:, :128], pt[:, :128], tri_sb)
                            pts.append((tkb, pt, nN, col0))
                        br_pts.append(pts)
                    # --- den (ones-matmuls) + PV; PE: den0, PV0, den1, PV1 ---
                    # full-width exp tiles are pair-summed on DVE first so the
                    # denominator needs ~30% fewer PE matmul cycles.
                    o_ps = {}
                    den_ps = {}
                    for br in range(2):
                        full = [p for p in br_pts[br] if p[2] == 512]
                        rest = [p for p in br_pts[br] if p[2] < 512]
                        items = []
                        for j in range(0, len(full) - 1, 2):
                            pp = sbB.tile([128, 512], BF16, tag="pp",
                                          bufs=8, name=f"pp{br}{j}")
                            nc.vector.tensor_add(pp, full[j][1], full[j + 1][1])
                            items.append((pp, 512, 0))
                        if len(full) % 2:
                            items.append((full[-1][1], 512, 0))
                        items += [(pt[:, :nN], nN, col0)
                                  for (tkb, pt, nN, col0) in rest]
                        dp = psS.tile([1, 512], F32, tag="s", name=f"den{br}")
                        for i, (rhs, nN, col0) in enumerate(items):
                            nc.tensor.matmul(
                                dp[:, col0:], ones_sb, rhs[:, :nN],
                                start=(i == 0), stop=(i == len(items) - 1))
                        den_ps[br] = dp
                        o_ps[br] = [psO.tile([128, 512], F32, tag="o",
                                             name=f"o_ps{br}{e}")
                                    for e in range(2)]
                        for e in range(2):
                            ecol = h * 256 + e * 128
                            for i, (tkb, pt, nN, col0) in enumerate(br_pts[br]):
                                nc.tensor.matmul(
                                    o_ps[br][e][:, col0:],
                                    v_all[:, tkb, ecol:ecol + 128],
                                    pt[:, :nN],
                                    start=(i == 0), stop=(i == ntk - 1))
                    # --- 1/den (fast approx), broadcast, combine ---
                    invb = {}
                    for br in range(2):
                        inv = sbB.tile([1, 512], F32, tag="inv",
                                       name=f"inv{br}")
                        nc.vector.reciprocal_approx_fast(inv, den_ps[br])
                        ib = sbB.tile([128, 512], F32, tag="invb",
                                      name=f"invb{br}")
                        nc.gpsimd.partition_broadcast(ib, inv)
                        invb[br] = ib
                    for e in range(2):
                        seg = h * 2 + e
                        o1n = sbB.tile([128, 512], F32, tag="o1n")
                        o2n = sbB.tile([128, 512], F32, tag="o2n")
                        nc.vector.tensor_mul(o1n, o_ps[0][e], invb[0])
                        nc.vector.tensor_mul(o2n, o_ps[1][e], invb[1])
                        of32 = sbB.tile([128, 512], F32, tag="of32")
                        nc.vector.scalar_tensor_tensor(
                            of32, o2n, neglam_sb, o1n,
                            mybir.AluOpType.mult, mybir.AluOpType.add)
                        # on-chip hi/lo e4m3 quantize for the fp8 out-proj
                        nc.vector.tensor_scalar(
                            oT_hi[:, seg, c0:c1], of32, SO, 0.0,
                            mybir.AluOpType.mult, mybir.AluOpType.add)
                        nc.vector.scalar_tensor_tensor(
                            oT_lo[:, seg, c0:c1], of32, SO,
                            oT_hi[:, seg, c0:c1],
                            mybir.AluOpType.mult, mybir.AluOpType.subtract)

        # ================= Phase C: out projection =================
        with ExitStack() as cctx:
            psY = cctx.enter_context(
                tc.tile_pool(name="psY", bufs=8, space="PSUM"))
            sbY = cctx.enter_context(tc.tile_pool(name="sbY", bufs=2))
            oT = [oT_hi, oT_lo]
            for m in range(NM):
                mlo, mhi = m * 128, (m + 1) * 128
                y_ps = [psY.tile([128, 512], F32, tag="y", name=f"y_ps{cc}")
                        for cc in range(4)]
                for ti, (oi, wi) in enumerate(((0, 0), (1, 0), (0, 1))):
                    for kp in range(2):
                        lhs = oT[oi][:, 2 * kp:2 * kp + 2, mlo:mhi]
                        st = ti == 0 and kp == 0
                        sp = ti == 2 and kp == 1
                        for cc in range(4):
                            nc.tensor.matmul(
                                y_ps[cc], lhs,
                                wo_sb[wi][:, 2 * kp:2 * kp + 2,
                                          cc * 512:(cc + 1) * 512],
                                start=st, stop=sp, perf_mode=DR)
                ystage = sbY.tile([128, T], BF16, tag="ystage")
                for cc in range(4):
                    if cc % 2 == 0:
                        nc.vector.tensor_scalar(
                            ystage[:, cc * 512:(cc + 1) * 512], y_ps[cc],
                            YSCALE, 0.0,
                            mybir.AluOpType.mult, mybir.AluOpType.add)
                    else:
                        nc.scalar.activation(
                            ystage[:, cc * 512:(cc + 1) * 512], y_ps[cc],
                            mybir.ActivationFunctionType.Copy, scale=YSCALE)
                nc.sync.dma_start(out=y[mlo:mhi, :], in_=ystage)


def build_nc():
    try:
        _setup_act_tables()
    except Exception:
        pass  # fall back to default tables (correct, extra table loads)
    nc = bacc.Bacc("TRN2", target_bir_lowering=False, debug=False,
                   num_devices=8)
    xTh = nc.dram_tensor("xTh", [C, T], FP8, kind="ExternalInput").ap()
    xTl = nc.dram_tensor("xTl", [C, T], FP8, kind="ExternalInput").ap()
    wqTh = nc.dram_tensor("wqTh", [C, 512], FP8, kind="ExternalInput").ap()
    wqTl = nc.dram_tensor("wqTl", [C, 512], FP8, kind="ExternalInput").ap()
    wkTh = nc.dram_tensor("wkTh", [C, 512], FP8, kind="ExternalInput").ap()
    wkTl = nc.dram_tensor("wkTl", [C, 512], FP8, kind="ExternalInput").ap()
    wvTh = nc.dram_tensor("wvTh", [C, 512], FP8, kind="ExternalInput").ap()
    wvTl = nc.dram_tensor("wvTl", [C, 512], FP8, kind="ExternalInput").ap()
    woTh = nc.dram_tensor("woTh", [512, C], FP8, kind="ExternalInput").ap()
    woTl = nc.dram_tensor("woTl", [512, C], FP8, kind="ExternalInput").ap()
    tri = nc.dram_tensor("tri", [128, 128], BF16, kind="ExternalInput").ap()
    ones = nc.dram_tensor("ones", [128, 1], BF16, kind="ExternalInput").ap()
    neglam = nc.dram_tensor("neglam", [128, 1], F32,
                            kind="ExternalInput").ap()
    y = nc.dram_tensor("y", [T, C], BF16, kind="ExternalOutput").ap()
    with tile.TileContext(nc) as tc:
        _body(tc, (xTh, xTl, wqTh, wqTl, wkTh, wkTl, wvTh, wvTl,
                   woTh, woTl, tri, ones, neglam, y))
    nc.compile()
    return nc


def _hilo(a):
    """Split f32 array into e4m3 hi + e4m3 residual."""
    hi = a.astype(NPFP8)
    lo = (a - hi.astype(np.float32)).astype(NPFP8)
    return hi, lo


def _host_prep(x, wq, wk, wv, wo, lq1, lk1, lq2, lk2):
    x = np.asarray(x, np.float32)
    wq, wk, wv, wo = (np.asarray(w, np.float32) for w in (wq, wk, wv, wo))
    lam = float(np.exp(np.sum(np.asarray(lq1, np.float32) *
                              np.asarray(lk1, np.float32))) -
                np.exp(np.sum(np.asarray(lq2, np.float32) *
                              np.asarray(lk2, np.float32))) + LAMBDA_INIT)

    tri = np.triu(np.ones((128, 128), np.float32)).astype(NPBF16)
    ones = np.ones((128, 1), np.float32).astype(NPBF16)
    neglam = np.full((128, 1), -lam, np.float32)

    in_maps = []
    for core in range(8):
        b = core // 4
        hp = core % 4
        h0, h1 = 2 * hp, 2 * hp + 1
        rows = np.r_[h0 * 256:(h0 + 1) * 256, h1 * 256:(h1 + 1) * 256]
        xTh, xTl = _hilo(np.ascontiguousarray(x[b].T) * SX)
        wqTh, wqTl = _hilo(np.ascontiguousarray(wq[rows, :].T) * SW)
        wkTh, wkTl = _hilo(np.ascontiguousarray(wk[rows, :].T) * SW)
        wvTh, wvTl = _hilo(np.ascontiguousarray(wv[rows, :].T) * SW)
        woTh, woTl = _hilo(np.ascontiguousarray(wo[:, rows].T) *
                           ((1.0 - LAMBDA_INIT) * SWO))
        in_maps.append({
            "xTh": xTh, "xTl": xTl,
            "wqTh": wqTh, "wqTl": wqTl,
            "wkTh": wkTh, "wkTl": wkTl,
            "wvTh": wvTh, "wvTl": wvTl,
            "woTh": woTh, "woTl": woTl,
            "tri": tri,
            "ones": ones,
            "neglam": neglam,
        })
    return in_maps


def kernel(x, wq, wk, wv, wo, lq1, lk1, lq2, lk2, _results_out=None,
           _trace=False):
    in_maps = _host_prep(x, wq, wk, wv, wo, lq1, lk1, lq2, lk2)
    nc = build_nc()
    res = bass_utils.run_bass_kernel_spmd(nc, in_maps,
                                          core_ids=list(range(8)),
                                          trace=_trace)
    if _results_out is not None:
        _results_out.append(res)
    out = np.zeros((B, T, C), np.float32)
    for core in range(8):
        out[core // 4] += res.results[core]["y"].astype(np.float32)
    return out


# revision 5
# speedup vs baseline: 1.3844x; 1.3844x over previous
"""DifferentialAttention Trainium2 Bass kernel.

Sharding: 8 cores = 2 batches x 4 head-pairs (2 heads each).
Per core (SPMD, same program, different data):
  phase A: q/k/v projections (bf16 matmuls, f32 PSUM) + RMS-norm + PE
           transpose into [d, t] layout. The reference's "rotary" uses the
           head index as its position, giving a constant orthogonal rotation
           per head identical for q and k -- it cancels in q.k^T and is
           skipped entirely. x streamed in 512-column waves so the m-loop
           never starves behind one monolithic tail DMA.
  phase B+C fused, chunk-major: differential causal attention per 512-token
           query chunk (exp without max-subtract; softmax denominator via
           ones-matmuls; per-branch output normalized and freed early so two
           PSUM banks suffice), with the previous chunk's output-projection
           m-tiles issued between S and PV as PE filler for the exp waits,
           spreading the y DMA across the whole phase. y emitted bf16;
           per-core partial y summed over head-pair cores on host.

fp8 DoubleRow was tried and reverted: it bursts at 2x for ~15us, then the
power manager throttles the PE to a 50% utilization limit, capping the
sustained MAC rate at exactly the bf16 rate (throttle_activity_1 17.6%,
util limit 0.5) -- while costing 1.5x the instructions for hi/lo accuracy.
"""

import json
import os
import sys
import tempfile
from contextlib import ExitStack

import numpy as np

sys.path.insert(0, "/opt/trn_rl_repo")

import ml_dtypes  # noqa: E402

import concourse.bass as bass  # noqa: E402
import concourse.mybir as mybir  # noqa: E402
import concourse.tile as tile  # noqa: E402
from concourse import bacc, bass_utils  # noqa: E402
from concourse.masks import make_identity  # noqa: E402

B, T, C = 2, 2048, 2048
NH, HD, HH = 8, 256, 128
LAMBDA_INIT = 0.2
RMS_EPS = 1.1920929e-07
SCALE = float(1.0 / np.sqrt(np.float32(HH)))

F32 = mybir.dt.float32
BF16 = mybir.dt.bfloat16
NPBF16 = ml_dtypes.bfloat16

NM = T // 128          # 16 m-tiles (t blocks)
NK = C // 128          # 16 k-tiles (c blocks)
NCH = T // 512         # 4 tq chunks

_ACT_TABLES_DONE = False


def _setup_act_tables():
    """Reorder act_info so `natural_log_exp_and_others` is the first table:
    it covers every ACT func we use (square, ln, exp, copy), so the greedy
    table selector stays on one table instead of thrashing exp<->ln loads."""
    global _ACT_TABLES_DONE
    if _ACT_TABLES_DONE:
        return
    from neuronxcc.driver.Job import Job  # noqa: PLC0415
    from neuronxcc.driver.jobs.support.FindActInfo import (  # noqa: PLC0415
        findActInfoFile,
    )

    src = findActInfoFile(Job.getPackageDir(), "gen3")
    srcdir = os.path.dirname(src)
    with open(src) as f:
        info = json.load(f)
    info["act_func_sets"].sort(
        key=lambda s: s["name"] != "natural_log_exp_and_others")
    dstdir = os.path.join(tempfile.gettempdir(), "act_info_nlexp_first")
    os.makedirs(dstdir, exist_ok=True)
    for name in os.listdir(srcdir):
        dst = os.path.join(dstdir, name)
        if not os.path.exists(dst):
            try:
                os.symlink(os.path.join(srcdir, name), dst)
            except OSError:
                pass
    act_path = os.path.join(dstdir, "act_info.json")
    with open(act_path, "w") as f:
        json.dump(info, f)
    os.environ["BASS_ACT_ROOT_JSON_PATH"] = act_path

    import concourse.hw_specs as hw_specs  # noqa: PLC0415

    def patched(module_arch):
        return {
            e["name"]: {
                mybir.ActivationFunctionType.from_pwp(v) for v in e["act"]
            }
            for e in info["act_func_sets"]
        }

    hw_specs.get_activation_tables = patched
    bacc.get_activation_tables = patched
    _ACT_TABLES_DONE = True


def _bcast_cols(ap2d, col0, nblk, inner):
    """[128, nblk, inner] view of columns col0..col0+nblk of a [128, n] tile,
    each column replicated `inner` times along a 0-stride inner dim."""
    return bass.AP(
        tensor=ap2d.tensor,
        offset=ap2d.offset + col0,
        ap=[ap2d.ap[0], [1, nblk], [0, inner]],
    )


def _body(tc, aps):
    nc = tc.nc
    xT, wqT, wkT, wvT, woT, tri, ones, neglam, y = aps

    xT_r = xT.rearrange("(k p) t -> p k t", p=128)      # [128, 16, 2048]
    wqT_r = wqT.rearrange("(k p) n -> p k n", p=128)    # [128, 16, 512]
    wkT_r = wkT.rearrange("(k p) n -> p k n", p=128)
    wvT_r = wvT.rearrange("(k p) n -> p k n", p=128)
    woT_r = woT.rearrange("(k p) n -> p k n", p=128)    # [128, 4, 2048]

    with ExitStack() as octx:
        # ---- persistent tiles (live across phases) ----
        persist = octx.enter_context(tc.tile_pool(name="persist", bufs=1))
        qkT_all = persist.tile([128, 8, T], BF16)    # seg h*4 + [q1,q2,k1,k2]
        v_all = persist.tile([128, NM, 512], BF16)   # [t(P) per m, e: h0|h1]
        tri_sb = persist.tile([128, 128], BF16)
        ones_sb = persist.tile([128, 1], BF16)
        neglam_sb = persist.tile([128, 1], F32)
        ident_sb = persist.tile([128, 128], BF16)
        oT_all = persist.tile([128, 4, T], BF16)     # seg h*2 + etile
        wo_sb = persist.tile([128, 4, T], BF16)

        nc.sync.dma_start(out=tri_sb, in_=tri)
        nc.sync.dma_start(out=ones_sb, in_=ones)
        nc.sync.dma_start(out=neglam_sb, in_=neglam)
        make_identity(nc, ident_sb)

        # segment mapping: qkT_all viewed [128, h, 4, T]; q -> [:, :, 0:2],
        # k -> [:, :, 2:4]; block order within a group is (h0b1,h0b2,h1b1,h1b2)
        qkT_v = qkT_all.rearrange("p (h f) t -> p h f t", h=2)

        # ================= Phase A: projections =================
        with ExitStack() as actx:
            wpool = actx.enter_context(tc.tile_pool(name="wpool", bufs=1))
            xpool = actx.enter_context(tc.tile_pool(name="xpool", bufs=1))
            psA = actx.enter_context(
                tc.tile_pool(name="psA", bufs=6, space="PSUM"))
            psT = actx.enter_context(
                tc.tile_pool(name="psT", bufs=2, space="PSUM"))
            sbA = actx.enter_context(tc.tile_pool(name="sbA", bufs=2))

            wq_sb = wpool.tile([128, NK, 512], BF16)
            wk_sb = wpool.tile([128, NK, 512], BF16)
            wv_sb = wpool.tile([128, NK, 512], BF16)
            x_sb = xpool.tile([128, NK, T], BF16)
            # wave 1: weights + the first 512 t-cols of x (everything the
            # m=0..3 tiles need), interleaved per kk so m=0 can trickle.
            for kk in range(NK):
                nc.sync.dma_start(out=x_sb[:, kk, 0:512],
                                  in_=xT_r[:, kk, 0:512])
                nc.sync.dma_start(out=wq_sb[:, kk, :], in_=wqT_r[:, kk, :])
                nc.sync.dma_start(out=wk_sb[:, kk, :], in_=wkT_r[:, kk, :])
                nc.sync.dma_start(out=wv_sb[:, kk, :], in_=wvT_r[:, kk, :])
            # x tail in 512-column waves: m=4(c+1).. tiles unblock as each
            # wave lands instead of waiting for one monolithic tail DMA.
            for c in range(1, 4):
                for kk in range(NK):
                    nc.sync.dma_start(
                        out=x_sb[:, kk, c * 512:(c + 1) * 512],
                        in_=xT_r[:, kk, c * 512:(c + 1) * 512])
                if c == 1:
                    for kk in range(4):
                        nc.sync.dma_start(out=wo_sb[:, kk, :],
                                          in_=woT_r[:, kk, :])

            # HAM warmup: identity matmuls fill the DMA-wait head so the PE
            # reaches K=8/8 before the first projection matmul. No DMA deps
            # (identity is gpsimd-generated); the psum scratch slot releases
            # before m=2 needs it.
            wu_ps = psA.tile([128, 128], F32, tag="acc")
            for _ in range(48):
                nc.tensor.matmul(wu_ps, ident_sb, ident_sb,
                                 start=True, stop=True)

            for m in range(NM):
                mlo, mhi = m * 128, (m + 1) * 128
                q_ps = psA.tile([128, 512], F32, tag="acc")
                k_ps = psA.tile([128, 512], F32, tag="acc")
                v_ps = psA.tile([128, 512], F32, tag="acc")
                for kk in range(NK):
                    lhs = x_sb[:, kk, mlo:mhi]
                    st, sp = kk == 0, kk == NK - 1
                    nc.tensor.matmul(q_ps, lhs, wq_sb[:, kk, :], start=st, stop=sp)
                    nc.tensor.matmul(k_ps, lhs, wk_sb[:, kk, :], start=st, stop=sp)
                    nc.tensor.matmul(v_ps, lhs, wv_sb[:, kk, :], start=st, stop=sp)

                # vacate PSUM quickly: stage q/k to SBUF, v straight out
                qs = sbA.tile([128, 512], F32, tag="qs")
                nc.scalar.copy(qs, q_ps)
                ks = sbA.tile([128, 512], F32, tag="ks")
                nc.vector.tensor_copy(ks, k_ps)
                nc.scalar.copy(v_all[:, m, :], v_ps)

                # sumsq of the 8 blocks: stats cols 0-3 = q, 4-7 = k
                stats = sbA.tile([128, 8], F32, tag="stats")
                sq_scr = sbA.tile([128, 128], F32, tag="sqscr")
                for j in range(8):
                    src = qs if j < 4 else ks
                    off = (j % 4) * 128
                    nc.scalar.activation(
                        sq_scr, src[:, off:off + 128],
                        mybir.ActivationFunctionType.Square,
                        accum_out=stats[:, j:j + 1])
                # rstd = exp(-0.5*ln(mean+eps)) -- ln/exp/square share a table
                veps = sbA.tile([128, 8], F32, tag="veps")
                nc.vector.tensor_scalar(
                    veps, stats, 1.0 / HH, RMS_EPS,
                    mybir.AluOpType.mult, mybir.AluOpType.add)
                lnv = sbA.tile([128, 8], F32, tag="lnv")
                nc.scalar.activation(
                    lnv, veps, mybir.ActivationFunctionType.Ln)
                rstd8 = sbA.tile([128, 8], F32, tag="rstd8")
                nc.scalar.activation(
                    rstd8, lnv, mybir.ActivationFunctionType.Exp, scale=-0.5)

                # normalize (no rotary: the reference's head-axis rotary is a
                # per-head constant orthogonal rotation shared by q and k, so
                # it cancels in q.k^T), batched over the 4 blocks of q then k
                for g, (src, c0) in enumerate(((qs, 0), (ks, 4))):
                    qnr = sbA.tile([128, 512], BF16, tag="qnr")
                    nc.vector.tensor_tensor(
                        qnr.rearrange("p (b i) -> p b i", b=4),
                        src.rearrange("p (b i) -> p b i", b=4),
                        _bcast_cols(rstd8, c0, 4, 128), mybir.AluOpType.mult)

                    tp = psT.tile([128, 512], BF16, tag="tp")
                    for bi in range(4):
                        nc.tensor.transpose(
                            tp[:, bi * 128:(bi + 1) * 128],
                            qnr[:, bi * 128:(bi + 1) * 128], ident_sb)
                    # dst: [128, h(2), br(2), 128] at segment group g
                    nc.vector.tensor_copy(
                        qkT_v[:, :, 2 * g:2 * g + 2, mlo:mhi],
                        tp.rearrange("p (h b i) -> p h b i", h=2, b=2))

        # ================= Phase B+C fused =================
        with ExitStack() as bctx:
            psS = bctx.enter_context(
                tc.tile_pool(name="psS", bufs=4, space="PSUM"))
            psO = bctx.enter_context(
                tc.tile_pool(name="psO", bufs=2, space="PSUM"))
            psY = bctx.enter_context(
                tc.tile_pool(name="psY", bufs=2, space="PSUM"))
            sbPT = bctx.enter_context(tc.tile_pool(name="sbPT", bufs=34))
            sbB = bctx.enter_context(tc.tile_pool(name="sbB", bufs=4))
            sbY = bctx.enter_context(tc.tile_pool(name="sbY", bufs=2))

            def outproj_mtile(m):
                mlo, mhi = m * 128, (m + 1) * 128
                ystage = sbY.tile([128, T], BF16, tag="ystage",
                                  name=f"ystage{m}")
                for cc in range(4):
                    y_ps = psY.tile([128, 512], F32, tag="y",
                                    name=f"y_ps{m}_{cc}")
                    for kk in range(4):
                        nc.tensor.matmul(
                            y_ps, oT_all[:, kk, mlo:mhi],
                            wo_sb[:, kk, cc * 512:(cc + 1) * 512],
                            start=(kk == 0), stop=(kk == 3))
                    if cc % 2 == 0:
                        nc.vector.tensor_copy(
                            ystage[:, cc * 512:(cc + 1) * 512], y_ps)
                    else:
                        nc.scalar.copy(
                            ystage[:, cc * 512:(cc + 1) * 512], y_ps)
                nc.sync.dma_start(out=y[mlo:mhi, :], in_=ystage)

            ready_outproj = []
            for ch in range(NCH):
                c0, c1 = ch * 512, ch * 512 + 512
                ntk = 4 * ch + 4
                for h in range(2):
                    qT = [qkT_v[:, h, 0, :], qkT_v[:, h, 1, :]]
                    kT = [qkT_v[:, h, 2, :], qkT_v[:, h, 3, :]]
                    # --- S + exp, both branches ---
                    br_pts = []
                    for br in range(2):
                        pts = []
                        for tkb in range(ntk):
                            n0 = max(c0, tkb * 128)
                            nN = c1 - n0
                            col0 = n0 - c0
                            s_ps = psS.tile([128, 512], F32, tag="s")
                            nc.tensor.matmul(
                                s_ps[:, :nN],
                                kT[br][:, tkb * 128:(tkb + 1) * 128],
                                qT[br][:, n0:c1], start=True, stop=True)
                            pt = sbPT.tile([128, 512], BF16, tag="pt")
                            nc.scalar.activation(
                                pt[:, :nN], s_ps[:, :nN],
                                mybir.ActivationFunctionType.Exp, scale=SCALE)
                            if tkb * 128 >= c0:   # diagonal block
                                nc.vector.tensor_mul(
                                    pt[:, :128], pt[:, :128], tri_sb)
                            pts.append((tkb, pt, nN, col0))
                        br_pts.append(pts)
                    # --- PE filler while ACT chews the exps: out-projection
                    # m-tiles of the previous chunk ---
                    for _ in range(2):
                        if ready_outproj:
                            outproj_mtile(ready_outproj.pop(0))
                    # --- den + PV per branch; normalize early so 2 PSUM
                    # o-banks suffice ---
                    onorm = [[None, None], [None, None]]
                    for br in range(2):
                        full = [p for p in br_pts[br] if p[2] == 512]
                        rest = [p for p in br_pts[br] if p[2] < 512]
                        items = []
                        for j in range(0, len(full) - 1, 2):
                            pp = sbB.tile([128, 512], BF16, tag="pp",
                                          bufs=8, name=f"pp{br}{j}")
                            nc.vector.tensor_add(pp, full[j][1], full[j + 1][1])
                            items.append((pp, 512, 0))
                        if len(full) % 2:
                            items.append((full[-1][1], 512, 0))
                        items += [(pt[:, :nN], nN, col0)
                                  for (tkb, pt, nN, col0) in rest]
                        dp = psS.tile([1, 512], F32, tag="s", name=f"den{br}")
                        for i, (rhs, nN, col0) in enumerate(items):
                            nc.tensor.matmul(
                                dp[:, col0:], ones_sb, rhs[:, :nN],
                                start=(i == 0), stop=(i == len(items) - 1))
                        inv = sbB.tile([1, 512], F32, tag="inv",
                                       name=f"inv{br}")
                        nc.vector.reciprocal_approx_fast(inv, dp)
                        ib = sbB.tile([128, 512], F32, tag="invb",
                                      name=f"invb{br}")
                        nc.gpsimd.partition_broadcast(ib, inv)
                        for e in range(2):
                            ecol = h * 256 + e * 128
                            o_ps = psO.tile([128, 512], F32, tag="o",
                                            name=f"o_ps{br}{e}")
                            for i, (tkb, pt, nN, col0) in enumerate(br_pts[br]):
                                nc.tensor.matmul(
                                    o_ps[:, col0:],
                                    v_all[:, tkb, ecol:ecol + 128],
                                    pt[:, :nN],
                                    start=(i == 0), stop=(i == ntk - 1))
                            on = sbB.tile([128, 512], F32, tag=f"on{br}",
                                          name=f"on{br}{e}")
                            nc.vector.tensor_mul(on, o_ps, ib)
                            onorm[br][e] = on
                    for e in range(2):
                        nc.vector.scalar_tensor_tensor(
                            oT_all[:, h * 2 + e, c0:c1], onorm[1][e],
                            neglam_sb, onorm[0][e],
                            mybir.AluOpType.mult, mybir.AluOpType.add)
                ready_outproj += [4 * ch + i for i in range(4)]
            for m in ready_outproj:
                outproj_mtile(m)


def build_nc():
    try:
        _setup_act_tables()
    except Exception:
        pass  # fall back to default tables (correct, extra table loads)
    nc = bacc.Bacc("TRN2", target_bir_lowering=False, debug=False,
                   num_devices=8)
    xT = nc.dram_tensor("xT", [C, T], BF16, kind="ExternalInput").ap()
    wqT = nc.dram_tensor("wqT", [C, 512], BF16, kind="ExternalInput").ap()
    wkT = nc.dram_tensor("wkT", [C, 512], BF16, kind="ExternalInput").ap()
    wvT = nc.dram_tensor("wvT", [C, 512], BF16, kind="ExternalInput").ap()
    woT = nc.dram_tensor("woT", [512, C], BF16, kind="ExternalInput").ap()
    tri = nc.dram_tensor("tri", [128, 128], BF16, kind="ExternalInput").ap()
    ones = nc.dram_tensor("ones", [128, 1], BF16, kind="ExternalInput").ap()
    neglam = nc.dram_tensor("neglam", [128, 1], F32,
                            kind="ExternalInput").ap()
    y = nc.dram_tensor("y", [T, C], BF16, kind="ExternalOutput").ap()
    with tile.TileContext(nc) as tc:
        _body(tc, (xT, wqT, wkT, wvT, woT, tri, ones, neglam, y))
    nc.compile()
    return nc


def _host_prep(x, wq, wk, wv, wo, lq1, lk1, lq2, lk2):
    x = np.asarray(x, np.float32)
    wq, wk, wv, wo = (np.asarray(w, np.float32) for w in (wq, wk, wv, wo))
    lam = float(np.exp(np.sum(np.asarray(lq1, np.float32) *
                              np.asarray(lk1, np.float32))) -
                np.exp(np.sum(np.asarray(lq2, np.float32) *
                              np.asarray(lk2, np.float32))) + LAMBDA_INIT)

    tri = np.triu(np.ones((128, 128), np.float32)).astype(NPBF16)
    ones = np.ones((128, 1), np.float32).astype(NPBF16)
    neglam = np.full((128, 1), -lam, np.float32)

    in_maps = []
    for core in range(8):
        b = core // 4
        hp = core % 4
        h0, h1 = 2 * hp, 2 * hp + 1
        rows = np.r_[h0 * 256:(h0 + 1) * 256, h1 * 256:(h1 + 1) * 256]
        in_maps.append({
            "xT": np.ascontiguousarray(x[b].T).astype(NPBF16),
            "wqT": np.ascontiguousarray(wq[rows, :].T).astype(NPBF16),
            "wkT": np.ascontiguousarray(wk[rows, :].T).astype(NPBF16),
            "wvT": np.ascontiguousarray(wv[rows, :].T).astype(NPBF16),
            "woT": np.ascontiguousarray(
                (wo[:, rows].T * (1.0 - LAMBDA_INIT))).astype(NPBF16),
            "tri": tri,
            "ones": ones,
            "neglam": neglam,
        })
    return in_maps


def kernel(x, wq, wk, wv, wo, lq1, lk1, lq2, lk2, _results_out=None,
           _trace=False):
    in_maps = _host_prep(x, wq, wk, wv, wo, lq1, lk1, lq2, lk2)
    nc = build_nc()
    res = bass_utils.run_bass_kernel_spmd(nc, in_maps,
                                          core_ids=list(range(8)),
                                          trace=_trace)
    if _results_out is not None:
        _results_out.append(res)
    out = np.zeros((B, T, C), np.float32)
    for core in range(8):
        out[core // 4] += res.results[core]["y"].astype(np.float32)
    return out


# revision 14
# speedup vs baseline: 1.4372x; 1.0381x over previous
"""DifferentialAttention Trainium2 Bass kernel.

Sharding: 8 cores = 2 batches x 4 head-pairs (2 heads each).
Per core (SPMD, same program, different data):
  phase A: q/k/v projections (bf16 matmuls, f32 PSUM) + RMS-norm + PE
           transpose into [d, t] layout. The reference's "rotary" uses the
           head index as its position, giving a constant orthogonal rotation
           per head identical for q and k -- it cancels in q.k^T and is
           skipped entirely. x streamed in 512-column waves so the m-loop
           never starves behind one monolithic tail DMA.
  phase B+C fused, chunk-major: differential causal attention per 512-token
           query chunk (exp without max-subtract; softmax denominator via
           ones-matmuls; per-branch output normalized and freed early so two
           PSUM banks suffice), with the previous chunk's output-projection
           m-tiles issued between S and PV as PE filler for the exp waits,
           spreading the y DMA across the whole phase. y emitted bf16;
           per-core partial y summed over head-pair cores on host.

fp8 DoubleRow was tried and reverted: it bursts at 2x for ~15us, then the
power manager throttles the PE to a 50% utilization limit, capping the
sustained MAC rate at exactly the bf16 rate (throttle_activity_1 17.6%,
util limit 0.5) -- while costing 1.5x the instructions for hi/lo accuracy.
"""

import json
import os
import sys
import tempfile
from contextlib import ExitStack

import numpy as np

sys.path.insert(0, "/opt/trn_rl_repo")

import ml_dtypes  # noqa: E402

import concourse.bass as bass  # noqa: E402
import concourse.mybir as mybir  # noqa: E402
import concourse.tile as tile  # noqa: E402
from concourse import bacc, bass_utils  # noqa: E402
from concourse.masks import make_identity  # noqa: E402

B, T, C = 2, 2048, 2048
NH, HD, HH = 8, 256, 128
LAMBDA_INIT = 0.2
RMS_EPS = 1.1920929e-07
SCALE = float(1.0 / np.sqrt(np.float32(HH)))

F32 = mybir.dt.float32
BF16 = mybir.dt.bfloat16
NPBF16 = ml_dtypes.bfloat16

NM = T // 128          # 16 m-tiles (t blocks)
NK = C // 128          # 16 k-tiles (c blocks)
NCH = T // 512         # 4 tq chunks

_ACT_TABLES_DONE = False


def _setup_act_tables():
    """Reorder act_info so `natural_log_exp_and_others` is the first table:
    it covers every ACT func we use (square, ln, exp, copy), so the greedy
    table selector stays on one table instead of thrashing exp<->ln loads."""
    global _ACT_TABLES_DONE
    if _ACT_TABLES_DONE:
        return
    from neuronxcc.driver.Job import Job  # noqa: PLC0415
    from neuronxcc.driver.jobs.support.FindActInfo import (  # noqa: PLC0415
        findActInfoFile,
    )

    src = findActInfoFile(Job.getPackageDir(), "gen3")
    srcdir = os.path.dirname(src)
    with open(src) as f:
        info = json.load(f)
    info["act_func_sets"].sort(
        key=lambda s: s["name"] != "natural_log_exp_and_others")
    dstdir = os.path.join(tempfile.gettempdir(), "act_info_nlexp_first")
    os.makedirs(dstdir, exist_ok=True)
    for name in os.listdir(srcdir):
        dst = os.path.join(dstdir, name)
        if not os.path.exists(dst):
            try:
                os.symlink(os.path.join(srcdir, name), dst)
            except OSError:
                pass
    act_path = os.path.join(dstdir, "act_info.json")
    with open(act_path, "w") as f:
        json.dump(info, f)
    os.environ["BASS_ACT_ROOT_JSON_PATH"] = act_path

    import concourse.hw_specs as hw_specs  # noqa: PLC0415

    def patched(module_arch):
        return {
            e["name"]: {
                mybir.ActivationFunctionType.from_pwp(v) for v in e["act"]
            }
            for e in info["act_func_sets"]
        }

    hw_specs.get_activation_tables = patched
    bacc.get_activation_tables = patched
    _ACT_TABLES_DONE = True


def _bcast_cols(ap2d, col0, nblk, inner):
    """[128, nblk, inner] view of columns col0..col0+nblk of a [128, n] tile,
    each column replicated `inner` times along a 0-stride inner dim."""
    return bass.AP(
        tensor=ap2d.tensor,
        offset=ap2d.offset + col0,
        ap=[ap2d.ap[0], [1, nblk], [0, inner]],
    )


def _body(tc, aps):
    nc = tc.nc
    xT, wqT, wkT, wvT, woT, tri, ones, neglam, y = aps

    xT_r = xT.rearrange("(k p) t -> p k t", p=128)      # [128, 16, 2048]
    wqT_r = wqT.rearrange("(k p) n -> p k n", p=128)    # [128, 16, 512]
    wkT_r = wkT.rearrange("(k p) n -> p k n", p=128)
    wvT_r = wvT.rearrange("(k p) n -> p k n", p=128)
    woT_r = woT.rearrange("(k p) n -> p k n", p=128)    # [128, 4, 2048]

    with ExitStack() as octx:
        # ---- persistent tiles (live across phases) ----
        persist = octx.enter_context(tc.tile_pool(name="persist", bufs=1))
        qkT_all = persist.tile([128, 8, T], BF16)    # seg h*4 + [q1,q2,k1,k2]
        v_all = persist.tile([128, NM, 512], BF16)   # [t(P) per m, e: h0|h1]
        tri_sb = persist.tile([128, 128], BF16)
        ones_sb = persist.tile([128, 1], BF16)
        neglam_sb = persist.tile([128, 1], F32)
        ident_sb = persist.tile([128, 128], BF16)

        nc.sync.dma_start(out=tri_sb, in_=tri)
        nc.sync.dma_start(out=ones_sb, in_=ones)
        nc.sync.dma_start(out=neglam_sb, in_=neglam)
        make_identity(nc, ident_sb)

        # segment mapping: qkT_all viewed [128, h, 4, T]; q -> [:, :, 0:2],
        # k -> [:, :, 2:4]; block order within a group is (h0b1,h0b2,h1b1,h1b2)
        qkT_v = qkT_all.rearrange("p (h f) t -> p h f t", h=2)

        # ================= Phase A: projections =================
        with ExitStack() as actx:
            wpool = actx.enter_context(tc.tile_pool(name="wpool", bufs=1))
            xpool = actx.enter_context(tc.tile_pool(name="xpool", bufs=1))
            psA = actx.enter_context(
                tc.tile_pool(name="psA", bufs=6, space="PSUM"))
            psT = actx.enter_context(
                tc.tile_pool(name="psT", bufs=2, space="PSUM"))
            sbA = actx.enter_context(tc.tile_pool(name="sbA", bufs=2))

            wq_sb = wpool.tile([128, NK, 512], BF16)
            wk_sb = wpool.tile([128, NK, 512], BF16)
            wv_sb = wpool.tile([128, NK, 512], BF16)
            x_sb = xpool.tile([128, NK, T], BF16)
            # DMA order matches the m0-3 prologue's per-projection passes:
            # wq + x wave 1 (the q-pass critical set), then wk, wv, x tail in
            # 512-column waves so m-tiles unblock wave by wave, then wo.
            for kk in range(NK):
                nc.sync.dma_start(out=wq_sb[:, kk, :], in_=wqT_r[:, kk, :])
                nc.sync.dma_start(out=x_sb[:, kk, 0:512],
                                  in_=xT_r[:, kk, 0:512])
            for kk in range(NK):
                nc.sync.dma_start(out=wk_sb[:, kk, :], in_=wkT_r[:, kk, :])
            for kk in range(NK):
                nc.sync.dma_start(out=wv_sb[:, kk, :], in_=wvT_r[:, kk, :])
            for c in range(1, 4):
                for kk in range(NK):
                    nc.sync.dma_start(
                        out=x_sb[:, kk, c * 512:(c + 1) * 512],
                        in_=xT_r[:, kk, c * 512:(c + 1) * 512])


            # HAM warmup: identity matmuls fill the DMA-wait head so the PE
            # reaches K=8/8 before the first projection matmul. No DMA deps
            # (identity is gpsimd-generated); the psum scratch slot releases
            # before the prologue needs it.
            wu_ps = psA.tile([128, 128], F32, tag="acc")
            for _ in range(48):
                nc.tensor.matmul(wu_ps, ident_sb, ident_sb,
                                 start=True, stop=True)

            def proj_mm(ps, w_sb, m):
                mlo, mhi = m * 128, (m + 1) * 128
                for kk in range(NK):
                    nc.tensor.matmul(ps, x_sb[:, kk, mlo:mhi],
                                     w_sb[:, kk, :],
                                     start=(kk == 0), stop=(kk == NK - 1))

            def norm_chain(m, qs, ks):
                """RMS stats + normalize; returns the two qnr tiles whose PE
                transposes the caller defers off the m-loop critical path."""
                stats = sbA.tile([128, 8], F32, tag="stats",
                                 name=f"stats{m}")
                sq_scr = sbA.tile([128, 128], F32, tag="sqscr",
                                  name=f"sqscr{m}")
                for j in range(8):
                    src = qs if j < 4 else ks
                    off = (j % 4) * 128
                    nc.scalar.activation(
                        sq_scr, src[:, off:off + 128],
                        mybir.ActivationFunctionType.Square,
                        accum_out=stats[:, j:j + 1])
                # rstd = exp(-0.5*ln(mean+eps)); ln/exp/square share a table
                veps = sbA.tile([128, 8], F32, tag="veps", name=f"veps{m}")
                nc.vector.tensor_scalar(
                    veps, stats, 1.0 / HH, RMS_EPS,
                    mybir.AluOpType.mult, mybir.AluOpType.add)
                lnv = sbA.tile([128, 8], F32, tag="lnv", name=f"lnv{m}")
                nc.scalar.activation(
                    lnv, veps, mybir.ActivationFunctionType.Ln)
                rstd8 = sbA.tile([128, 8], F32, tag="rstd8",
                                 name=f"rstd8{m}")
                nc.scalar.activation(
                    rstd8, lnv, mybir.ActivationFunctionType.Exp, scale=-0.5)

                # normalize (no rotary: the reference's head-axis rotary is a
                # per-head constant orthogonal rotation shared by q and k, so
                # it cancels in q.k^T), batched over the 4 blocks of q then k
                qnrs = []
                for g, (src, c0) in enumerate(((qs, 0), (ks, 4))):
                    qnr = sbA.tile([128, 512], BF16, tag="qnr", bufs=4,
                                   name=f"qnr{m}{g}")
                    nc.vector.tensor_tensor(
                        qnr.rearrange("p (b i) -> p b i", b=4),
                        src.rearrange("p (b i) -> p b i", b=4),
                        _bcast_cols(rstd8, c0, 4, 128), mybir.AluOpType.mult)
                    qnrs.append(qnr)
                return qnrs

            def transposes(m, qnrs):
                mlo, mhi = m * 128, (m + 1) * 128
                for g, qnr in enumerate(qnrs):
                    tp = psT.tile([128, 512], BF16, tag="tp")
                    for bi in range(4):
                        nc.tensor.transpose(
                            tp[:, bi * 128:(bi + 1) * 128],
                            qnr[:, bi * 128:(bi + 1) * 128], ident_sb)
                    # dst: [128, h(2), br(2), 128] at segment group g
                    nc.vector.tensor_copy(
                        qkT_v[:, :, 2 * g:2 * g + 2, mlo:mhi],
                        tp.rearrange("p (h b i) -> p h b i", h=2, b=2))

            # --- prologue m0-3: per-projection passes so the q-pass starts
            # after only wq + x wave 1 (~4MB) instead of the full 8.3MB ---
            pro_qs, pending = [], []
            for m in range(4):
                q_ps = psA.tile([128, 512], F32, tag="acc", name=f"pq{m}")
                proj_mm(q_ps, wq_sb, m)
                qs = sbA.tile([128, 512], F32, tag="qs", bufs=4,
                              name=f"qs{m}")
                nc.scalar.copy(qs, q_ps)
                pro_qs.append(qs)
            for m in range(4):
                k_ps = psA.tile([128, 512], F32, tag="acc", name=f"pk{m}")
                proj_mm(k_ps, wk_sb, m)
                ks = sbA.tile([128, 512], F32, tag="ks", bufs=2,
                              name=f"ks{m}")
                nc.vector.tensor_copy(ks, k_ps)
                pending.append((m, norm_chain(m, pro_qs[m], ks)))
            for m in range(4):
                v_ps = psA.tile([128, 512], F32, tag="acc", name=f"pv{m}")
                proj_mm(v_ps, wv_sb, m)
                nc.scalar.copy(v_all[:, m, :], v_ps)
                transposes(*pending.pop(0))

            # --- steady state m4+: q/k/v interleaved per kk; transposes of
            # m-1 issued after m's matmuls so the PE never waits on the
            # ACT/DVE norm chain ---
            for m in range(4, NM):
                mlo, mhi = m * 128, (m + 1) * 128
                q_ps = psA.tile([128, 512], F32, tag="acc")
                k_ps = psA.tile([128, 512], F32, tag="acc")
                v_ps = psA.tile([128, 512], F32, tag="acc")
                for kk in range(NK):
                    lhs = x_sb[:, kk, mlo:mhi]
                    st, sp = kk == 0, kk == NK - 1
                    nc.tensor.matmul(q_ps, lhs, wq_sb[:, kk, :], start=st, stop=sp)
                    nc.tensor.matmul(k_ps, lhs, wk_sb[:, kk, :], start=st, stop=sp)
                    nc.tensor.matmul(v_ps, lhs, wv_sb[:, kk, :], start=st, stop=sp)

                while pending:
                    transposes(*pending.pop(0))

                # vacate PSUM quickly: stage q/k to SBUF, v straight out
                qs = sbA.tile([128, 512], F32, tag="qs", bufs=4)
                nc.scalar.copy(qs, q_ps)
                ks = sbA.tile([128, 512], F32, tag="ks", bufs=2)
                nc.vector.tensor_copy(ks, k_ps)
                nc.scalar.copy(v_all[:, m, :], v_ps)
                pending.append((m, norm_chain(m, qs, ks)))
            while pending:
                transposes(*pending.pop(0))

        # ================= Phase B+C fused =================
        with ExitStack() as bctx:
            bcpool = bctx.enter_context(tc.tile_pool(name="bcpool", bufs=1))
            oT_all = bcpool.tile([128, 4, T], BF16)  # seg h*2 + etile
            wo_sb = bcpool.tile([128, 4, T], BF16)
            # wo lands during ch1's attention, well before the first filler
            for kk in range(4):
                nc.sync.dma_start(out=wo_sb[:, kk, :], in_=woT_r[:, kk, :])
            psS = bctx.enter_context(
                tc.tile_pool(name="psS", bufs=4, space="PSUM"))
            psO = bctx.enter_context(
                tc.tile_pool(name="psO", bufs=2, space="PSUM"))
            psY = bctx.enter_context(
                tc.tile_pool(name="psY", bufs=2, space="PSUM"))
            sbPT = bctx.enter_context(tc.tile_pool(name="sbPT", bufs=34))
            sbB = bctx.enter_context(tc.tile_pool(name="sbB", bufs=4))
            sbY = bctx.enter_context(tc.tile_pool(name="sbY", bufs=2))

            # out-projection issued one 512-col cc-group at a time so single
            # groups can be woven into every attention stall point
            ready_cc = []          # FIFO of (m, cc)
            ystages = {}

            def pop_filler(n):
                for _ in range(n):
                    if not ready_cc:
                        return
                    m, cc = ready_cc.pop(0)
                    mlo, mhi = m * 128, (m + 1) * 128
                    if cc == 0:
                        ystages[m] = sbY.tile([128, T], BF16, tag="ystage",
                                              name=f"ystage{m}")
                    ystage = ystages[m]
                    y_ps = psY.tile([128, 512], F32, tag="y",
                                    name=f"y_ps{m}_{cc}")
                    for kk in range(4):
                        nc.tensor.matmul(
                            y_ps, oT_all[:, kk, mlo:mhi],
                            wo_sb[:, kk, cc * 512:(cc + 1) * 512],
                            start=(kk == 0), stop=(kk == 3))
                    if cc % 2 == 0:
                        nc.vector.tensor_copy(
                            ystage[:, cc * 512:(cc + 1) * 512], y_ps)
                    else:
                        nc.scalar.copy(
                            ystage[:, cc * 512:(cc + 1) * 512], y_ps)
                    if cc == 3:
                        nc.sync.dma_start(out=y[mlo:mhi, :], in_=ystage)
                        del ystages[m]

            # ch1 first: its out-projection work becomes the PE filler for
            # the exp-bound ch0; ch0's fills ch2, etc.
            for ch in (1, 0, 2, 3):
                c0, c1 = ch * 512, ch * 512 + 512
                ntk = 4 * ch + 4
                for h in range(2):
                    qT = [qkT_v[:, h, 0, :], qkT_v[:, h, 1, :]]
                    kT = [qkT_v[:, h, 2, :], qkT_v[:, h, 3, :]]
                    # --- S + exp, both branches, fillers woven between ---
                    br_pts = []
                    for br in range(2):
                        pts = []
                        for tkb in range(ntk):
                            n0 = max(c0, tkb * 128)
                            nN = c1 - n0
                            col0 = n0 - c0
                            s_ps = psS.tile([128, 512], F32, tag="s")
                            nc.tensor.matmul(
                                s_ps[:, :nN],
                                kT[br][:, tkb * 128:(tkb + 1) * 128],
                                qT[br][:, n0:c1], start=True, stop=True)
                            pt = sbPT.tile([128, 512], BF16, tag="pt")
                            nc.scalar.activation(
                                pt[:, :nN], s_ps[:, :nN],
                                mybir.ActivationFunctionType.Exp, scale=SCALE)
                            if tkb * 128 >= c0:   # diagonal block
                                nc.vector.tensor_mul(
                                    pt[:, :128], pt[:, :128], tri_sb)
                            pts.append((tkb, pt, nN, col0))
                        br_pts.append(pts)
                        pop_filler(1 + ntk // 4)
                    # --- den + PV per branch; normalize early so 2 PSUM
                    # o-banks suffice ---
                    onorm = [[None, None], [None, None]]
                    for br in range(2):
                        full = [p for p in br_pts[br] if p[2] == 512]
                        rest = [p for p in br_pts[br] if p[2] < 512]
                        items = []
                        for j in range(0, len(full) - 1, 2):
                            pp = sbB.tile([128, 512], BF16, tag="pp",
                                          bufs=8, name=f"pp{br}{j}")
                            nc.vector.tensor_add(pp, full[j][1], full[j + 1][1])
                            items.append((pp, 512, 0))
                        if len(full) % 2:
                            items.append((full[-1][1], 512, 0))
                        items += [(pt[:, :nN], nN, col0)
                                  for (tkb, pt, nN, col0) in rest]
                        dp = psS.tile([1, 512], F32, tag="s", name=f"den{br}")
                        for i, (rhs, nN, col0) in enumerate(items):
                            nc.tensor.matmul(
                                dp[:, col0:], ones_sb, rhs[:, :nN],
                                start=(i == 0), stop=(i == len(items) - 1))
                        inv = sbB.tile([1, 512], F32, tag="inv",
                                       name=f"inv{br}")
                        nc.vector.reciprocal_approx_fast(inv, dp)
                        ib = sbB.tile([128, 512], F32, tag="invb",
                                      name=f"invb{br}")
                        nc.gpsimd.partition_broadcast(ib, inv)
                        for e in range(2):
                            ecol = h * 256 + e * 128
                            o_ps = psO.tile([128, 512], F32, tag="o",
                                            name=f"o_ps{br}{e}")
                            for i, (tkb, pt, nN, col0) in enumerate(br_pts[br]):
                                nc.tensor.matmul(
                                    o_ps[:, col0:],
                                    v_all[:, tkb, ecol:ecol + 128],
                                    pt[:, :nN],
                                    start=(i == 0), stop=(i == ntk - 1))
                            on = sbB.tile([128, 512], F32, tag=f"on{br}",
                                          name=f"on{br}{e}")
                            nc.vector.tensor_mul(on, o_ps, ib)
                            onorm[br][e] = on
                            pop_filler(1)
                    for e in range(2):
                        nc.vector.scalar_tensor_tensor(
                            oT_all[:, h * 2 + e, c0:c1], onorm[1][e],
                            neglam_sb, onorm[0][e],
                            mybir.AluOpType.mult, mybir.AluOpType.add)
                ready_cc += [(4 * ch + i, cc) for i in range(4)
                             for cc in range(4)]
            pop_filler(len(ready_cc))


def build_nc():
    try:
        _setup_act_tables()
    except Exception:
        pass  # fall back to default tables (correct, extra table loads)
    nc = bacc.Bacc("TRN2", target_bir_lowering=False, debug=False,
                   num_devices=8)
    xT = nc.dram_tensor("xT", [C, T], BF16, kind="ExternalInput").ap()
    wqT = nc.dram_tensor("wqT", [C, 512], BF16, kind="ExternalInput").ap()
    wkT = nc.dram_tensor("wkT", [C, 512], BF16, kind="ExternalInput").ap()
    wvT = nc.dram_tensor("wvT", [C, 512], BF16, kind="ExternalInput").ap()
    woT = nc.dram_tensor("woT", [512, C], BF16, kind="ExternalInput").ap()
    tri = nc.dram_tensor("tri", [128, 128], BF16, kind="ExternalInput").ap()
    ones = nc.dram_tensor("ones", [128, 1], BF16, kind="ExternalInput").ap()
    neglam = nc.dram_tensor("neglam", [128, 1], F32,
                            kind="ExternalInput").ap()
    y = nc.dram_tensor("y", [T, C], BF16, kind="ExternalOutput").ap()
    with tile.TileContext(nc) as tc:
        _body(tc, (xT, wqT, wkT, wvT, woT, tri, ones, neglam, y))
    nc.compile()
    return nc


def _host_prep(x, wq, wk, wv, wo, lq1, lk1, lq2, lk2):
    x = np.asarray(x, np.float32)
    wq, wk, wv, wo = (np.asarray(w, np.float32) for w in (wq, wk, wv, wo))
    lam = float(np.exp(np.sum(np.asarray(lq1, np.float32) *
                              np.asarray(lk1, np.float32))) -
                np.exp(np.sum(np.asarray(lq2, np.float32) *
                              np.asarray(lk2, np.float32))) + LAMBDA_INIT)

    tri = np.triu(np.ones((128, 128), np.float32)).astype(NPBF16)
    ones = np.ones((128, 1), np.float32).astype(NPBF16)
    neglam = np.full((128, 1), -lam, np.float32)

    in_maps = []
    for core in range(8):
        b = core // 4
        hp = core % 4
        h0, h1 = 2 * hp, 2 * hp + 1
        rows = np.r_[h0 * 256:(h0 + 1) * 256, h1 * 256:(h1 + 1) * 256]
        in_maps.append({
            "xT": np.ascontiguousarray(x[b].T).astype(NPBF16),
            "wqT": np.ascontiguousarray(wq[rows, :].T).astype(NPBF16),
            "wkT": np.ascontiguousarray(wk[rows, :].T).astype(NPBF16),
            "wvT": np.ascontiguousarray(wv[rows, :].T).astype(NPBF16),
            "woT": np.ascontiguousarray(
                (wo[:, rows].T * (1.0 - LAMBDA_INIT))).astype(NPBF16),
            "tri": tri,
            "ones": ones,
            "neglam": neglam,
        })
    return in_maps


def kernel(x, wq, wk, wv, wo, lq1, lk1, lq2, lk2, _results_out=None,
           _trace=False):
    in_maps = _host_prep(x, wq, wk, wv, wo, lq1, lk1, lq2, lk2)
    nc = build_nc()
    res = bass_utils.run_bass_kernel_spmd(nc, in_maps,
                                          core_ids=list(range(8)),
                                          trace=_trace)
    if _results_out is not None:
        _results_out.append(res)
    out = np.zeros((B, T, C), np.float32)
    for core in range(8):
        out[core // 4] += res.results[core]["y"].astype(np.float32)
    return out


# revision 16
# speedup vs baseline: 1.4437x; 1.0045x over previous
"""DifferentialAttention Trainium2 Bass kernel.

Sharding: 8 cores = 2 batches x 4 head-pairs (2 heads each).
Per core (SPMD, same program, different data):
  phase A: q/k/v projections (bf16 matmuls, f32 PSUM) + RMS-norm + PE
           transpose into [d, t] layout. The reference's "rotary" uses the
           head index as its position, giving a constant orthogonal rotation
           per head identical for q and k -- it cancels in q.k^T and is
           skipped entirely. x streamed in 512-column waves so the m-loop
           never starves behind one monolithic tail DMA.
  phase B+C fused, chunk-major: differential causal attention per 512-token
           query chunk (exp without max-subtract; softmax denominator via
           ones-matmuls; per-branch output normalized and freed early so two
           PSUM banks suffice), with the previous chunk's output-projection
           m-tiles issued between S and PV as PE filler for the exp waits,
           spreading the y DMA across the whole phase. y emitted bf16;
           per-core partial y summed over head-pair cores on host.

fp8 DoubleRow was tried and reverted: it bursts at 2x for ~15us, then the
power manager throttles the PE to a 50% utilization limit, capping the
sustained MAC rate at exactly the bf16 rate (throttle_activity_1 17.6%,
util limit 0.5) -- while costing 1.5x the instructions for hi/lo accuracy.
"""

import json
import os
import sys
import tempfile
from contextlib import ExitStack

import numpy as np

sys.path.insert(0, "/opt/trn_rl_repo")

import ml_dtypes  # noqa: E402

import concourse.bass as bass  # noqa: E402
import concourse.mybir as mybir  # noqa: E402
import concourse.tile as tile  # noqa: E402
from concourse import bacc, bass_utils  # noqa: E402
from concourse.masks import make_identity  # noqa: E402

B, T, C = 2, 2048, 2048
NH, HD, HH = 8, 256, 128
LAMBDA_INIT = 0.2
RMS_EPS = 1.1920929e-07
SCALE = float(1.0 / np.sqrt(np.float32(HH)))

F32 = mybir.dt.float32
BF16 = mybir.dt.bfloat16
NPBF16 = ml_dtypes.bfloat16

NM = T // 128          # 16 m-tiles (t blocks)
NK = C // 128          # 16 k-tiles (c blocks)
NCH = T // 512         # 4 tq chunks

_ACT_TABLES_DONE = False


def _setup_act_tables():
    """Reorder act_info so `natural_log_exp_and_others` is the first table:
    it covers every ACT func we use (square, ln, exp, copy), so the greedy
    table selector stays on one table instead of thrashing exp<->ln loads."""
    global _ACT_TABLES_DONE
    if _ACT_TABLES_DONE:
        return
    from neuronxcc.driver.Job import Job  # noqa: PLC0415
    from neuronxcc.driver.jobs.support.FindActInfo import (  # noqa: PLC0415
        findActInfoFile,
    )

    src = findActInfoFile(Job.getPackageDir(), "gen3")
    srcdir = os.path.dirname(src)
    with open(src) as f:
        info = json.load(f)
    info["act_func_sets"].sort(
        key=lambda s: s["name"] != "natural_log_exp_and_others")
    dstdir = os.path.join(tempfile.gettempdir(), "act_info_nlexp_first")
    os.makedirs(dstdir, exist_ok=True)
    for name in os.listdir(srcdir):
        dst = os.path.join(dstdir, name)
        if not os.path.exists(dst):
            try:
                os.symlink(os.path.join(srcdir, name), dst)
            except OSError:
                pass
    act_path = os.path.join(dstdir, "act_info.json")
    with open(act_path, "w") as f:
        json.dump(info, f)
    os.environ["BASS_ACT_ROOT_JSON_PATH"] = act_path

    import concourse.hw_specs as hw_specs  # noqa: PLC0415

    def patched(module_arch):
        return {
            e["name"]: {
                mybir.ActivationFunctionType.from_pwp(v) for v in e["act"]
            }
            for e in info["act_func_sets"]
        }

    hw_specs.get_activation_tables = patched
    bacc.get_activation_tables = patched
    _ACT_TABLES_DONE = True


def _bcast_cols(ap2d, col0, nblk, inner):
    """[128, nblk, inner] view of columns col0..col0+nblk of a [128, n] tile,
    each column replicated `inner` times along a 0-stride inner dim."""
    return bass.AP(
        tensor=ap2d.tensor,
        offset=ap2d.offset + col0,
        ap=[ap2d.ap[0], [1, nblk], [0, inner]],
    )


def _body(tc, aps):
    nc = tc.nc
    xT, wqT, wkT, wvT, woT, tri, ones, neglam, y = aps

    xT_r = xT.rearrange("(k p) t -> p k t", p=128)      # [128, 16, 2048]
    wqT_r = wqT.rearrange("(k p) n -> p k n", p=128)    # [128, 16, 512]
    wkT_r = wkT.rearrange("(k p) n -> p k n", p=128)
    wvT_r = wvT.rearrange("(k p) n -> p k n", p=128)
    woT_r = woT.rearrange("(k p) n -> p k n", p=128)    # [128, 4, 2048]

    with ExitStack() as octx:
        # ---- persistent tiles (live across phases) ----
        persist = octx.enter_context(tc.tile_pool(name="persist", bufs=1))
        qkT_all = persist.tile([128, 8, T], BF16)    # seg h*4 + [q1,q2,k1,k2]
        v_all = persist.tile([128, NM, 512], BF16)   # [t(P) per m, e: h0|h1]
        tri_sb = persist.tile([128, 128], BF16)
        ones_sb = persist.tile([128, 1], BF16)
        neglam_sb = persist.tile([128, 1], F32)
        ident_sb = persist.tile([128, 128], BF16)

        nc.sync.dma_start(out=tri_sb, in_=tri)
        nc.sync.dma_start(out=ones_sb, in_=ones)
        nc.sync.dma_start(out=neglam_sb, in_=neglam)
        make_identity(nc, ident_sb)

        # segment mapping: qkT_all viewed [128, h, 4, T]; q -> [:, :, 0:2],
        # k -> [:, :, 2:4]; block order within a group is (h0b1,h0b2,h1b1,h1b2)
        qkT_v = qkT_all.rearrange("p (h f) t -> p h f t", h=2)

        # ================= Phase A: projections =================
        with ExitStack() as actx:
            wpool = actx.enter_context(tc.tile_pool(name="wpool", bufs=1))
            xpool = actx.enter_context(tc.tile_pool(name="xpool", bufs=1))
            psA = actx.enter_context(
                tc.tile_pool(name="psA", bufs=6, space="PSUM"))
            psT = actx.enter_context(
                tc.tile_pool(name="psT", bufs=2, space="PSUM"))
            sbA = actx.enter_context(tc.tile_pool(name="sbA", bufs=2))

            wq_sb = wpool.tile([128, NK, 512], BF16)
            wk_sb = wpool.tile([128, NK, 512], BF16)
            wv_sb = wpool.tile([128, NK, 512], BF16)
            x_sb = xpool.tile([128, NK, T], BF16)
            # DMA order matches the m0-3 prologue's per-projection passes:
            # wq + x wave 1 (the q-pass critical set), then wk, wv, x tail in
            # 512-column waves so m-tiles unblock wave by wave, then wo.
            for kk in range(NK):
                nc.sync.dma_start(out=wq_sb[:, kk, :], in_=wqT_r[:, kk, :])
                nc.sync.dma_start(out=x_sb[:, kk, 0:512],
                                  in_=xT_r[:, kk, 0:512])
            for kk in range(NK):
                nc.sync.dma_start(out=wk_sb[:, kk, :], in_=wkT_r[:, kk, :])
            for kk in range(NK):
                nc.sync.dma_start(out=wv_sb[:, kk, :], in_=wvT_r[:, kk, :])
            for c in range(1, 4):
                for kk in range(NK):
                    nc.sync.dma_start(
                        out=x_sb[:, kk, c * 512:(c + 1) * 512],
                        in_=xT_r[:, kk, c * 512:(c + 1) * 512])


            # HAM warmup: identity matmuls fill the DMA-wait head so the PE
            # reaches K=8/8 before the first projection matmul. No DMA deps
            # (identity is gpsimd-generated); the psum scratch slot releases
            # before the prologue needs it.
            wu_ps = psA.tile([128, 128], F32, tag="acc")
            for _ in range(48):
                nc.tensor.matmul(wu_ps, ident_sb, ident_sb,
                                 start=True, stop=True)

            def proj_mm(ps, w_sb, m):
                mlo, mhi = m * 128, (m + 1) * 128
                for kk in range(NK):
                    nc.tensor.matmul(ps, x_sb[:, kk, mlo:mhi],
                                     w_sb[:, kk, :],
                                     start=(kk == 0), stop=(kk == NK - 1))

            def norm_chain(m, qs, ks):
                """RMS stats + normalize; returns the two qnr tiles whose PE
                transposes the caller defers off the m-loop critical path."""
                stats = sbA.tile([128, 8], F32, tag="stats",
                                 name=f"stats{m}")
                sq_scr = sbA.tile([128, 128], F32, tag="sqscr",
                                  name=f"sqscr{m}")
                for j in range(8):
                    src = qs if j < 4 else ks
                    off = (j % 4) * 128
                    nc.scalar.activation(
                        sq_scr, src[:, off:off + 128],
                        mybir.ActivationFunctionType.Square,
                        accum_out=stats[:, j:j + 1])
                # rstd = exp(-0.5*ln(mean+eps)); ln/exp/square share a table
                veps = sbA.tile([128, 8], F32, tag="veps", name=f"veps{m}")
                nc.vector.tensor_scalar(
                    veps, stats, 1.0 / HH, RMS_EPS,
                    mybir.AluOpType.mult, mybir.AluOpType.add)
                lnv = sbA.tile([128, 8], F32, tag="lnv", name=f"lnv{m}")
                nc.scalar.activation(
                    lnv, veps, mybir.ActivationFunctionType.Ln)
                rstd8 = sbA.tile([128, 8], F32, tag="rstd8",
                                 name=f"rstd8{m}")
                nc.scalar.activation(
                    rstd8, lnv, mybir.ActivationFunctionType.Exp, scale=-0.5)

                # normalize (no rotary: the reference's head-axis rotary is a
                # per-head constant orthogonal rotation shared by q and k, so
                # it cancels in q.k^T), batched over the 4 blocks of q then k
                qnrs = []
                for g, (src, c0) in enumerate(((qs, 0), (ks, 4))):
                    qnr = sbA.tile([128, 512], BF16, tag="qnr", bufs=4,
                                   name=f"qnr{m}{g}")
                    nc.vector.tensor_tensor(
                        qnr.rearrange("p (b i) -> p b i", b=4),
                        src.rearrange("p (b i) -> p b i", b=4),
                        _bcast_cols(rstd8, c0, 4, 128), mybir.AluOpType.mult)
                    qnrs.append(qnr)
                return qnrs

            def transposes(m, qnrs):
                mlo, mhi = m * 128, (m + 1) * 128
                for g, qnr in enumerate(qnrs):
                    tp = psT.tile([128, 512], BF16, tag="tp")
                    for bi in range(4):
                        nc.tensor.transpose(
                            tp[:, bi * 128:(bi + 1) * 128],
                            qnr[:, bi * 128:(bi + 1) * 128], ident_sb)
                    # dst: [128, h(2), br(2), 128] at segment group g
                    nc.vector.tensor_copy(
                        qkT_v[:, :, 2 * g:2 * g + 2, mlo:mhi],
                        tp.rearrange("p (h b i) -> p h b i", h=2, b=2))

            # --- prologue m0-3: per-projection passes so the q-pass starts
            # after only wq + x wave 1 (~4MB) instead of the full 8.3MB ---
            # kk-major across the 4 m-tiles: per-kk PE work (4x213ns) slightly
            # exceeds the per-kk DMA (0.26MB), so the PE rides the q-pass
            # trickle with almost no idle.
            def proj_pass(w_sb, tag):
                pss = [psA.tile([128, 512], F32, tag="acc",
                                name=f"p{tag}{m}") for m in range(4)]
                for kk in range(NK):
                    for m in range(4):
                        nc.tensor.matmul(
                            pss[m], x_sb[:, kk, m * 128:(m + 1) * 128],
                            w_sb[:, kk, :],
                            start=(kk == 0), stop=(kk == NK - 1))
                return pss

            pending = []
            q_pss = proj_pass(wq_sb, "q")
            pro_qs = []
            for m in range(4):
                qs = sbA.tile([128, 512], F32, tag="qs", bufs=4,
                              name=f"qs{m}")
                nc.scalar.copy(qs, q_pss[m])
                pro_qs.append(qs)
            k_pss = proj_pass(wk_sb, "k")
            for m in range(4):
                ks = sbA.tile([128, 512], F32, tag="ks", bufs=2,
                              name=f"ks{m}")
                nc.vector.tensor_copy(ks, k_pss[m])
                pending.append((m, norm_chain(m, pro_qs[m], ks)))
            v_pss = proj_pass(wv_sb, "v")
            for m in range(4):
                nc.scalar.copy(v_all[:, m, :], v_pss[m])
                transposes(*pending.pop(0))

            # --- steady state m4+: q/k/v interleaved per kk; transposes of
            # m-1 issued after m's matmuls so the PE never waits on the
            # ACT/DVE norm chain ---
            for m in range(4, NM):
                mlo, mhi = m * 128, (m + 1) * 128
                q_ps = psA.tile([128, 512], F32, tag="acc")
                k_ps = psA.tile([128, 512], F32, tag="acc")
                v_ps = psA.tile([128, 512], F32, tag="acc")
                for kk in range(NK):
                    lhs = x_sb[:, kk, mlo:mhi]
                    st, sp = kk == 0, kk == NK - 1
                    nc.tensor.matmul(q_ps, lhs, wq_sb[:, kk, :], start=st, stop=sp)
                    nc.tensor.matmul(k_ps, lhs, wk_sb[:, kk, :], start=st, stop=sp)
                    nc.tensor.matmul(v_ps, lhs, wv_sb[:, kk, :], start=st, stop=sp)

                while pending:
                    transposes(*pending.pop(0))

                # vacate PSUM quickly: stage q/k to SBUF, v straight out
                qs = sbA.tile([128, 512], F32, tag="qs", bufs=4)
                nc.scalar.copy(qs, q_ps)
                ks = sbA.tile([128, 512], F32, tag="ks", bufs=2)
                nc.vector.tensor_copy(ks, k_ps)
                nc.scalar.copy(v_all[:, m, :], v_ps)
                pending.append((m, norm_chain(m, qs, ks)))
            while pending:
                transposes(*pending.pop(0))

        # ================= Phase B+C fused =================
        with ExitStack() as bctx:
            bcpool = bctx.enter_context(tc.tile_pool(name="bcpool", bufs=1))
            oT_all = bcpool.tile([128, 4, T], BF16)  # seg h*2 + etile
            wo_sb = bcpool.tile([128, 4, T], BF16)
            # wo lands during ch1's attention, well before the first filler
            for kk in range(4):
                nc.sync.dma_start(out=wo_sb[:, kk, :], in_=woT_r[:, kk, :])
            psS = bctx.enter_context(
                tc.tile_pool(name="psS", bufs=4, space="PSUM"))
            psO = bctx.enter_context(
                tc.tile_pool(name="psO", bufs=2, space="PSUM"))
            psY = bctx.enter_context(
                tc.tile_pool(name="psY", bufs=2, space="PSUM"))
            sbPT = bctx.enter_context(tc.tile_pool(name="sbPT", bufs=34))
            sbB = bctx.enter_context(tc.tile_pool(name="sbB", bufs=4))
            sbY = bctx.enter_context(tc.tile_pool(name="sbY", bufs=2))

            # out-projection issued one 512-col cc-group at a time so single
            # groups can be woven into every attention stall point
            ready_cc = []          # FIFO of (m, cc)
            ystages = {}

            def pop_filler(n):
                for _ in range(n):
                    if not ready_cc:
                        return
                    m, cc = ready_cc.pop(0)
                    mlo, mhi = m * 128, (m + 1) * 128
                    if cc == 0:
                        ystages[m] = sbY.tile([128, T], BF16, tag="ystage",
                                              name=f"ystage{m}")
                    ystage = ystages[m]
                    y_ps = psY.tile([128, 512], F32, tag="y",
                                    name=f"y_ps{m}_{cc}")
                    for kk in range(4):
                        nc.tensor.matmul(
                            y_ps, oT_all[:, kk, mlo:mhi],
                            wo_sb[:, kk, cc * 512:(cc + 1) * 512],
                            start=(kk == 0), stop=(kk == 3))
                    if cc % 2 == 0:
                        nc.vector.tensor_copy(
                            ystage[:, cc * 512:(cc + 1) * 512], y_ps)
                    else:
                        nc.scalar.copy(
                            ystage[:, cc * 512:(cc + 1) * 512], y_ps)
                    if cc == 3:
                        nc.sync.dma_start(out=y[mlo:mhi, :], in_=ystage)
                        del ystages[m]

            # ch1 first: its out-projection work becomes the PE filler for
            # the exp-bound ch0; ch0's fills ch2, etc.
            for ch in (1, 0, 2, 3):
                c0, c1 = ch * 512, ch * 512 + 512
                ntk = 4 * ch + 4
                for h in range(2):
                    qT = [qkT_v[:, h, 0, :], qkT_v[:, h, 1, :]]
                    kT = [qkT_v[:, h, 2, :], qkT_v[:, h, 3, :]]
                    # --- S + exp, both branches, fillers woven between ---
                    br_pts = []
                    for br in range(2):
                        pts = []
                        for tkb in range(ntk):
                            n0 = max(c0, tkb * 128)
                            nN = c1 - n0
                            col0 = n0 - c0
                            s_ps = psS.tile([128, 512], F32, tag="s")
                            nc.tensor.matmul(
                                s_ps[:, :nN],
                                kT[br][:, tkb * 128:(tkb + 1) * 128],
                                qT[br][:, n0:c1], start=True, stop=True)
                            pt = sbPT.tile([128, 512], BF16, tag="pt")
                            nc.scalar.activation(
                                pt[:, :nN], s_ps[:, :nN],
                                mybir.ActivationFunctionType.Exp, scale=SCALE)
                            if tkb * 128 >= c0:   # diagonal block
                                nc.vector.tensor_mul(
                                    pt[:, :128], pt[:, :128], tri_sb)
                            pts.append((tkb, pt, nN, col0))
                        br_pts.append(pts)
                        pop_filler(1 + ntk // 4)
                    # --- dens + reciprocal broadcasts for BOTH branches
                    # first: the gpsimd broadcast has multi-us wakeup
                    # latency, so it must be in flight before the PVs whose
                    # normalize step consumes it ---
                    ibs = []
                    for br in range(2):
                        full = [p for p in br_pts[br] if p[2] == 512]
                        rest = [p for p in br_pts[br] if p[2] < 512]
                        items = []
                        for j in range(0, len(full) - 1, 2):
                            pp = sbB.tile([128, 512], BF16, tag="pp",
                                          bufs=8, name=f"pp{br}{j}")
                            nc.vector.tensor_add(pp, full[j][1], full[j + 1][1])
                            items.append((pp, 512, 0))
                        if len(full) % 2:
                            items.append((full[-1][1], 512, 0))
                        items += [(pt[:, :nN], nN, col0)
                                  for (tkb, pt, nN, col0) in rest]
                        dp = psS.tile([1, 512], F32, tag="s", name=f"den{br}")
                        for i, (rhs, nN, col0) in enumerate(items):
                            nc.tensor.matmul(
                                dp[:, col0:], ones_sb, rhs[:, :nN],
                                start=(i == 0), stop=(i == len(items) - 1))
                        inv = sbB.tile([1, 512], F32, tag="inv",
                                       name=f"inv{br}")
                        nc.vector.reciprocal_approx_fast(inv, dp)
                        ib = sbB.tile([128, 512], F32, tag="invb",
                                      name=f"invb{br}")
                        nc.gpsimd.partition_broadcast(ib, inv)
                        ibs.append(ib)
                    pop_filler(1)
                    # --- PV per branch; normalize right after each PV so 2
                    # PSUM o-banks suffice ---
                    onorm = [[None, None], [None, None]]
                    for br in range(2):
                        for e in range(2):
                            ecol = h * 256 + e * 128
                            o_ps = psO.tile([128, 512], F32, tag="o",
                                            name=f"o_ps{br}{e}")
                            for i, (tkb, pt, nN, col0) in enumerate(br_pts[br]):
                                nc.tensor.matmul(
                                    o_ps[:, col0:],
                                    v_all[:, tkb, ecol:ecol + 128],
                                    pt[:, :nN],
                                    start=(i == 0), stop=(i == ntk - 1))
                            on = sbB.tile([128, 512], F32, tag=f"on{br}",
                                          name=f"on{br}{e}")
                            nc.vector.tensor_mul(on, o_ps, ibs[br])
                            onorm[br][e] = on
                            pop_filler(1)
                    for e in range(2):
                        nc.vector.scalar_tensor_tensor(
                            oT_all[:, h * 2 + e, c0:c1], onorm[1][e],
                            neglam_sb, onorm[0][e],
                            mybir.AluOpType.mult, mybir.AluOpType.add)
                ready_cc += [(4 * ch + i, cc) for i in range(4)
                             for cc in range(4)]
            pop_filler(len(ready_cc))


def build_nc():
    try:
        _setup_act_tables()
    except Exception:
        pass  # fall back to default tables (correct, extra table loads)
    nc = bacc.Bacc("TRN2", target_bir_lowering=False, debug=False,
                   num_devices=8)
    xT = nc.dram_tensor("xT", [C, T], BF16, kind="ExternalInput").ap()
    wqT = nc.dram_tensor("wqT", [C, 512], BF16, kind="ExternalInput").ap()
    wkT = nc.dram_tensor("wkT", [C, 512], BF16, kind="ExternalInput").ap()
    wvT = nc.dram_tensor("wvT", [C, 512], BF16, kind="ExternalInput").ap()
    woT = nc.dram_tensor("woT", [512, C], BF16, kind="ExternalInput").ap()
    tri = nc.dram_tensor("tri", [128, 128], BF16, kind="ExternalInput").ap()
    ones = nc.dram_tensor("ones", [128, 1], BF16, kind="ExternalInput").ap()
    neglam = nc.dram_tensor("neglam", [128, 1], F32,
                            kind="ExternalInput").ap()
    y = nc.dram_tensor("y", [T, C], BF16, kind="ExternalOutput").ap()
    with tile.TileContext(nc) as tc:
        _body(tc, (xT, wqT, wkT, wvT, woT, tri, ones, neglam, y))
    nc.compile()
    return nc


def _host_prep(x, wq, wk, wv, wo, lq1, lk1, lq2, lk2):
    x = np.asarray(x, np.float32)
    wq, wk, wv, wo = (np.asarray(w, np.float32) for w in (wq, wk, wv, wo))
    lam = float(np.exp(np.sum(np.asarray(lq1, np.float32) *
                              np.asarray(lk1, np.float32))) -
                np.exp(np.sum(np.asarray(lq2, np.float32) *
                              np.asarray(lk2, np.float32))) + LAMBDA_INIT)

    tri = np.triu(np.ones((128, 128), np.float32)).astype(NPBF16)
    ones = np.ones((128, 1), np.float32).astype(NPBF16)
    neglam = np.full((128, 1), -lam, np.float32)

    in_maps = []
    for core in range(8):
        b = core // 4
        hp = core % 4
        h0, h1 = 2 * hp, 2 * hp + 1
        rows = np.r_[h0 * 256:(h0 + 1) * 256, h1 * 256:(h1 + 1) * 256]
        in_maps.append({
            "xT": np.ascontiguousarray(x[b].T).astype(NPBF16),
            "wqT": np.ascontiguousarray(wq[rows, :].T).astype(NPBF16),
            "wkT": np.ascontiguousarray(wk[rows, :].T).astype(NPBF16),
            "wvT": np.ascontiguousarray(wv[rows, :].T).astype(NPBF16),
            "woT": np.ascontiguousarray(
                (wo[:, rows].T * (1.0 - LAMBDA_INIT))).astype(NPBF16),
            "tri": tri,
            "ones": ones,
            "neglam": neglam,
        })
    return in_maps


def kernel(x, wq, wk, wv, wo, lq1, lk1, lq2, lk2, _results_out=None,
           _trace=False):
    in_maps = _host_prep(x, wq, wk, wv, wo, lq1, lk1, lq2, lk2)
    nc = build_nc()
    res = bass_utils.run_bass_kernel_spmd(nc, in_maps,
                                          core_ids=list(range(8)),
                                          trace=_trace)
    if _results_out is not None:
        _results_out.append(res)
    out = np.zeros((B, T, C), np.float32)
    for core in range(8):
        out[core // 4] += res.results[core]["y"].astype(np.float32)
    return out
